# revision 21
# baseline (speedup 1.0000x reference)
"""BertCRF loss kernel for 8 Trainium2 NeuronCores.

Layout/algorithm:
  - Data-parallel over batch: core c handles sequences [8c, 8c+8).
  - Host pre-transposes hidden to hidT [768, 4096] per core so the
    contraction dim lands on SBUF partitions with contiguous DMA.
  - Device: logits^T = Wrep^T @ hidT via fp32r matmuls into PSUM [81, 512]
    chunks, where Wrep tiles W 9x so PSUM holds logits replicated 9x
    across partitions (q = 9i+k -> logit k). One Exp activation per chunk
    produces Erep = exp(logits + b) [81, 4096]; rows 0:9 give logits out.
  - CRF partition function: exp-space transfer matrices D_l = T*diag(E_l).
    Device builds per-sequence 8-step block products G_blk = D_{8b+1}..D_{8b+8}
    for blk 0..62 (steps 1..504), batched over all 504 (seq, blk) chains:
    one [81,81]x[81,504] fp32r matmul (block-diag kron(I9, expT)) + one
    tensor_mul per step. 8-step products stay within fp32 range.
  - Host (fp64): alpha scan over the 63 block matrices + 7 tail steps,
    numerator (gold path score), final loss. Tiny (O(B*L) flops).
"""

import sys

if "/opt/trn_rl_repo" not in sys.path:
    sys.path.insert(0, "/opt/trn_rl_repo")

import numpy as np

B, L, H, T = 64, 512, 768, 9
NCORES = 8
BPC = B // NCORES          # sequences per core
NPOS = BPC * L             # 4096 positions per core
CH = 8                     # position chunks for the matmul pipeline
CHW = NPOS // CH           # 512 positions per chunk
HC = H // 128              # 6 contraction chunks
Q = 81                     # replicated tag dim (9i+k)
CSTEP = 2                  # steps per CRF block
NBLK = 252                 # blocks per sequence (steps 1..504)
NCHAIN = BPC * NBLK        # 504 chains per core

_CACHE = {}


def _build_program(stage=3, CHUNK_PLAN=((0,1),(1,2),(3,2),(5,1),(6,1),(7,1))):
    import concourse.bacc as bacc
    import concourse.mybir as mybir
    from concourse.tile import TileContext

    f32 = mybir.dt.float32
    f32r = mybir.dt.float32r
    nc = bacc.Bacc("TRN2", target_bir_lowering=False)

    hidT = nc.dram_tensor("hidT", [H, NPOS], f32r, kind="ExternalInput")
    Wrep = nc.dram_tensor("Wrep", [H, Q], f32r, kind="ExternalInput")
    brep = nc.dram_tensor("brep", [Q, 1], f32, kind="ExternalInput")
    tt2c = nc.dram_tensor("tt2c", [T, Q], f32, kind="ExternalInput")
    logitsT_out = nc.dram_tensor("logitsT", [T, NPOS], f32, kind="ExternalOutput")
    G_out = nc.dram_tensor("G", [Q, NCHAIN], f32, kind="ExternalOutput")

    # (start_seq, n_seqs): small head chunk so compute starts early, small
    # tail chunk so the exposed post-DMA compute is short.
    CHUNKS = CHUNK_PLAN
    HHALF = HC // 2

    with TileContext(nc) as tc:
        with (
            tc.tile_pool(name="const", bufs=1) as cpool,
            tc.tile_pool(name="hid", bufs=4) as hpool,
            tc.tile_pool(name="acc", bufs=1) as apool,
            tc.tile_pool(name="ps", bufs=4, space="PSUM") as ppool,
            tc.tile_pool(name="gps", bufs=2, space="PSUM") as gppool,
        ):
            w_sb = cpool.tile([128, HC, Q], f32r)
            nc.sync.dma_start(
                out=w_sb[:, :, :],
                in_=Wrep[:, :].rearrange("(c p) m -> p c m", p=128),
            )
            brep_sb = cpool.tile([Q, 1], f32)
            nc.sync.dma_start(out=brep_sb[:, :], in_=brep[:, :])
            tt2_sb = cpool.tile([T, Q], f32)
            nc.sync.dma_start(out=tt2_sb[:, :], in_=tt2c[:, :])

            erep = apool.tile([Q, NPOS], f32)
            logits_sb = apool.tile([T, NPOS], f32)
            gall = apool.tile([Q, BPC, NBLK], f32)

            for b0, ns in CHUNKS:
                w = ns * L
                ht = hpool.tile([128, HC, 2 * L], f32r, name="ht")
                # split by h-halves so matmuls can start at half-chunk latency
                for hh in range(2):
                    nc.sync.dma_start(
                        out=ht[:, hh * HHALF : (hh + 1) * HHALF, :w],
                        in_=hidT[
                            hh * (H // 2) : (hh + 1) * (H // 2),
                            b0 * L : b0 * L + w,
                        ].rearrange("(c p) m -> p c m", p=128),
                    )
                if stage < 1:
                    continue
                for sub in range(ns):
                    b = b0 + sub
                    ps = ppool.tile([Q, CHW], f32)
                    for hc in range(HC):
                        nc.tensor.matmul(
                            ps[:, :],
                            w_sb[:, hc, :],
                            ht[:, hc, sub * CHW : (sub + 1) * CHW],
                            start=(hc == 0),
                            stop=(hc == HC - 1),
                        )
                    nc.scalar.activation(
                        erep[:, b * CHW : (b + 1) * CHW],
                        ps[:, :],
                        mybir.ActivationFunctionType.Exp,
                        bias=brep_sb[:, 0:1],
                    )
                    nc.vector.tensor_scalar_add(
                        logits_sb[:, b * CHW : (b + 1) * CHW],
                        ps[0:T, :],
                        brep_sb[0:T, 0:1],
                    )
                if stage < 2:
                    continue
                # CRF pair-block products for this chunk's sequences:
                # G_k[i,m] = (sum_j T[i,j]T[j,m] E_{2k+1}[j]) * E_{2k+2}[m]
                erep_c = erep[:, b0 * L : b0 * L + w].rearrange(
                    "q (s k m) -> q s k m", s=ns, k=L // CSTEP
                )
                gp = gppool.tile([Q, 2, NBLK], f32, name="gp")
                nc.tensor.matmul(
                    gp[:, :ns, :],
                    tt2_sb[:, :],
                    erep_c[0:T, :, 0:NBLK, 1],
                    start=True,
                    stop=True,
                )
                nc.vector.tensor_mul(
                    gall[:, b0 : b0 + ns, :],
                    gp[:, :ns, :],
                    erep_c[:, :, 1 : NBLK + 1, 0],
                )
                nc.scalar.dma_start(
                    out=logitsT_out[:, b0 * L : b0 * L + w],
                    in_=logits_sb[:, b0 * L : b0 * L + w],
                )
                nc.scalar.dma_start(
                    out=G_out[:, b0 * NBLK : (b0 + ns) * NBLK],
                    in_=gall[:, b0 : b0 + ns, :].rearrange("q b k -> q (b k)"),
                )

    nc.compile()
    return nc


def _get_program():
    if "nc" not in _CACHE:
        _CACHE["nc"] = _build_program()
    return _CACHE["nc"]


def _round_f32r(x):
    import ml_dtypes

    x = np.asarray(x, np.float32)
    hi = x.astype(ml_dtypes.bfloat16).astype(np.float32)
    lo = (x - hi).astype(ml_dtypes.bfloat16).astype(np.float32)
    return hi + lo


def _make_const_inputs(W, b, trans):
    W = np.asarray(W, np.float32)
    b = np.asarray(b, np.float32)
    expT = np.exp(np.asarray(trans, np.float64))            # [9,9] fp64
    Wrep = np.tile(W, (1, T)).astype(np.float32)            # [768, 81], col 9i+k = W[:,k]
    brep = np.tile(b, T).reshape(Q, 1).astype(np.float32)   # [81,1]
    # tt2c[j, 9i+m] = expT[i,j] * expT[j,m]
    tt2c = (expT.T[:, :, None] * expT[:, None, :]).reshape(T, Q).astype(np.float32)
    return _round_f32r(Wrep), brep, np.ascontiguousarray(tt2c)


def _logsumexp(x, axis):
    m = np.max(x, axis=axis, keepdims=True)
    return (m + np.log(np.sum(np.exp(x - m), axis=axis, keepdims=True))).squeeze(axis)


def _numerator(logits64, labels, mask, start_trans, end_trans, trans):
    maskf = mask.astype(np.float64)
    safe = np.where(labels == -100, 0, labels).astype(np.int64)
    first = safe[:, 0]
    num = start_trans[first] + logits64[np.arange(B), 0, first]
    emit = np.take_along_axis(logits64, safe[..., None], axis=2)[..., 0]
    step_t = trans[safe[:, :-1], safe[:, 1:]]
    num = num + np.sum((step_t + emit[:, 1:]) * maskf[:, 1:], axis=1)
    seq_ends = mask.sum(axis=1).astype(np.int64) - 1
    last = np.take_along_axis(safe, seq_ends[:, None], axis=1)[:, 0]
    return num + end_trans[last]


def _host_reference(hidden, attention_mask, labels, W, b, start_trans, end_trans, trans):
    """Pure-numpy fallback (general mask). fp64."""
    logits = hidden.astype(np.float64) @ W.astype(np.float64) + b.astype(np.float64)
    mask = attention_mask.astype(bool)
    st, et, tr = (np.asarray(x, np.float64) for x in (start_trans, end_trans, trans))
    num = _numerator(logits, labels, mask, st, et, tr)
    score = st[None, :] + logits[:, 0]
    for l in range(1, L):
        nxt = _logsumexp(score[:, :, None] + tr[None] + logits[:, l][:, None, :], axis=1)
        score = np.where(mask[:, l][:, None], nxt, score)
    denom = _logsumexp(score + et[None, :], axis=1)
    loss = -np.mean(num - denom)
    return np.float32(loss), logits.astype(np.float32)


def _make_in_maps(hidden, W, b, trans):
    Wrep, brep, tt2c = _make_const_inputs(W, b, trans)
    in_maps = []
    for c in range(NCORES):
        hs = hidden[c * BPC : (c + 1) * BPC].reshape(NPOS, H).astype(np.float32)
        hidT = _round_f32r(np.ascontiguousarray(hs.T))  # [768, 4096]
        in_maps.append({"hidT": hidT, "Wrep": Wrep, "brep": brep, "tt2c": tt2c})
    return in_maps


def kernel(hidden, attention_mask, labels, W, b, start_trans, end_trans, trans):
    from concourse.bass_utils import run_bass_kernel_spmd

    hidden = np.asarray(hidden)
    attention_mask = np.asarray(attention_mask)
    labels = np.asarray(labels)
    if not np.all(attention_mask == 1):
        return _host_reference(
            hidden, attention_mask, labels, W, b, start_trans, end_trans, trans
        )

    in_maps = _make_in_maps(hidden, W, b, trans)
    nc = _get_program()
    last_err = None
    for attempt in range(3):
        try:
            res = run_bass_kernel_spmd(nc, in_maps, core_ids=list(range(NCORES)))
            break
        except Exception as e:  # transient NRT/terminal failures
            last_err = e
            import time as _time

            _time.sleep(2.0 * (attempt + 1))
    else:
        raise last_err

    logits = np.empty((B, L, T), np.float32)
    G_all = np.empty((NCORES, Q, NCHAIN), np.float64)
    for c in range(NCORES):
        lt = res.results[c]["logitsT"]  # [9, 4096]
        logits[c * BPC : (c + 1) * BPC] = (
            lt.reshape(T, BPC, L).transpose(1, 2, 0)
        )
        G_all[c] = res.results[c]["G"].astype(np.float64)

    # ---- host combine (fp64) ----
    logits64 = logits.astype(np.float64)
    st, et, tr = (np.asarray(x, np.float64) for x in (start_trans, end_trans, trans))
    # logG[b_global, blk, i, k] from G[9i+k, bpc*63+blk]
    logG = np.log(
        G_all.reshape(NCORES, T, T, BPC, NBLK).transpose(0, 3, 4, 1, 2)
    ).reshape(B, NBLK, T, T)
    alpha = st[None, :] + logits64[:, 0]
    for blk in range(NBLK):
        alpha = _logsumexp(alpha[:, :, None] + logG[:, blk], axis=1)
    for l in range(NBLK * CSTEP + 1, L):
        alpha = _logsumexp(
            alpha[:, :, None] + tr[None] + logits64[:, l][:, None, :], axis=1
        )
    denom = _logsumexp(alpha + et[None, :], axis=1)
    num = _numerator(logits64, labels, attention_mask.astype(bool), st, et, tr)
    loss = -np.mean(num - denom)
    return np.float32(loss), logits
